# revision 1
# baseline (speedup 1.0000x reference)
"""GCN message-passing kernel for Trainium2 (8 NeuronCores, SPMD).

Problem: 5-layer GCN with densenet-style concat over a random graph
(N=100000 nodes, E=1600000 edges + self-loops, H=64).

Approach (nodes sharded by destination across 8 cores):
  x_i = relu(x_{i-1} + sum_j (A @ xs_j)^T-cached @ W_i[j] + b_i)
with A the symmetrically-normalized adjacency. Per layer, each core:
  - AllGathers the scaled features xs_{i-1} = x_{i-1} * dinv into a full
    [100352, 64] f32 gather table (layer 1 uses a host-provided table),
  - gathers per-edge source rows with dma_gather (int16 idx relative to a
    src-quarter base; edges sorted by (superblock, quarter, block)),
  - segment-sums messages into 128-dst-node PSUM blocks with one-hot
    matmuls on TensorE (one-hot built by per-partition is_equal on DVE),
  - scales by dinv, transposes (aggT cached in DRAM for later layers),
  - dense-combines cached aggT_j with W_i (+bias row), applies residual
    +relu, and produces the next xs shard for the AllGather.

Host side does graph preprocessing only: self-loops, degree/dinv,
edge sort + padding to 128-edge chunks, int16 index packing.
"""

import numpy as np

N = 100000
E_RAW = 1600000
H = 64
NCORES = 8
NS = 12500            # real nodes per core
NSP = 12544           # padded (98 * 128)
NPAD = NSP * NCORES   # 100352 padded global rows
P = 128
NBLK = NSP // P       # 98
QROWS = 32768         # rows addressable by int16 idx per gather call
NQ = 4
GBLK = 4              # blocks per superblock (PSUM-bank limited)
NSB = (NBLK + GBLK - 1) // GBLK  # 25
NLAYERS = 5

_COMPILED = None  # (nc, meta) cache


# ----------------------------------------------------------------------------
# host-side graph preprocessing
# ----------------------------------------------------------------------------

def _pad_id(n):
    return (n // NS) * NSP + (n % NS)


def _preprocess(edge_index):
    src = np.asarray(edge_index[0], dtype=np.int64)
    dst = np.asarray(edge_index[1], dtype=np.int64)
    loop = np.arange(N, dtype=np.int64)
    src = np.concatenate([src, loop])
    dst = np.concatenate([dst, loop])

    deg = np.bincount(dst, minlength=N).astype(np.float32)
    dinv = np.where(deg > 0, 1.0 / np.sqrt(np.maximum(deg, 1e-12)), 0.0)
    dinv = dinv.astype(np.float32)

    srcp = _pad_id(src)

    per_core = []
    for c in range(NCORES):
        m = (dst >= c * NS) & (dst < (c + 1) * NS)
        dl = (dst[m] - c * NS).astype(np.int64)
        sp = srcp[m]
        blk = dl // P
        q = sp // QROWS
        sb = blk // GBLK
        order = np.lexsort((blk, q, sb))
        dl, sp, blk, q = dl[order], sp[order], blk[order], q[order]
        # group boundaries on (blk, q) within the (sb, q, blk) sort
        per_core.append((dl, sp, blk, q))

    # per-(b,q) edge counts per core -> uniform capacities (in chunks)
    counts = np.zeros((NCORES, NBLK, NQ), np.int64)
    for c in range(NCORES):
        dl, sp, blk, q = per_core[c]
        np.add.at(counts[c], (blk, q), 1)
    cap = counts.max(axis=0)                      # [NBLK, NQ] edges
    capc = (cap + P - 1) // P                     # chunks, >= 0

    # static call/chunk schedule (same for every core); each dma_gather call
    # is capped at MAXCALL chunks (1024 descriptors = HW in-flight limit)
    MAXCALL = 8
    calls = []
    idx_off = 0
    ch_off = 0
    for s in range(NSB):
        bs = range(s * GBLK, min((s + 1) * GBLK, NBLK))
        for qq in range(NQ):
            # flat chunk list [(b, n_chunks_of_b_in_this_call)] split at MAXCALL
            pend = [(b, int(capc[b, qq])) for b in bs if capc[b, qq] > 0]
            while pend:
                blocks = []
                room = MAXCALL
                while pend and room > 0:
                    b, nb = pend[0]
                    take = min(nb, room)
                    blocks.append((b, take))
                    room -= take
                    if take == nb:
                        pend.pop(0)
                    else:
                        pend[0] = (b, nb - take)
                nch = sum(nb for _, nb in blocks)
                calls.append(dict(q=qq, nch=nch, idx_off=idx_off,
                                  ch_off=ch_off, blocks=blocks))
                idx_off += nch * P // 16
                ch_off += nch
    TOTC = ch_off
    TOTI = idx_off
    MAXCH = max(cl["nch"] for cl in calls)

    # block -> (last call, chunks per (b,q)) map for start/stop flags
    blk_tot_ch = capc.sum(axis=1)  # chunks per block

    # pack per-core idx / dstl
    idx_all = np.zeros((NCORES, P, TOTI), np.int16)
    dstl_all = np.full((NCORES, P, TOTC), 200.0, np.float32)
    for c in range(NCORES):
        dl, sp, blk, q = per_core[c]
        # padded per-(b,q) edge arrays at uniform capacity
        key = blk * NQ + q
        grp_idx = {}
        bounds = np.flatnonzero(np.diff(key)) + 1
        starts = np.concatenate([[0], bounds])
        ends = np.concatenate([bounds, [len(key)]])
        for st, en in zip(starts, ends):
            grp_idx[(int(blk[st]), int(q[st]))] = (st, en)
        padded = {}   # (b, q) -> (rel, dd) arrays of len capc*P
        for b in range(NBLK):
            for qq in range(NQ):
                nb = int(capc[b, qq])
                if nb == 0:
                    continue
                st, en = grp_idx.get((b, qq), (0, 0))
                n = en - st
                rel = np.zeros(nb * P, np.int64)
                dd = np.full(nb * P, 200, np.int64)
                if n:
                    rel[:n] = sp[st:en] - qq * QROWS
                    dd[:n] = dl[st:en] - b * P
                padded[(b, qq)] = (rel, dd)
        cursor = {}   # (b, q) -> chunks consumed so far
        for cl in calls:
            qq = cl["q"]
            io = cl["idx_off"]
            co = cl["ch_off"]
            rel_parts = []
            dst_parts = []
            for b, nb in cl["blocks"]:
                cur = cursor.get((b, qq), 0)
                rel, dd = padded[(b, qq)]
                rel_parts.append(rel[cur * P:(cur + nb) * P])
                dst_parts.append(dd[cur * P:(cur + nb) * P])
                cursor[(b, qq)] = cur + nb
            rel = np.concatenate(rel_parts)
            dd = np.concatenate(dst_parts)
            ncall = cl["nch"] * P
            w = rel.astype(np.int16).reshape(-1, 16).T      # [16, ncall/16]
            idx_all[c, :, io:io + ncall // 16] = np.tile(w, (8, 1))
            dstl_all[c, :, co:co + cl["nch"]] = (
                dd.astype(np.float32).reshape(-1, P).T)

    meta = dict(calls=calls, TOTC=TOTC, TOTI=TOTI, MAXCH=MAXCH,
                blk_tot_ch=blk_tot_ch)
    return meta, idx_all, dstl_all, dinv, src, dst


# ----------------------------------------------------------------------------
# device kernel builder
# ----------------------------------------------------------------------------

def _build_nc(meta):
    import concourse.bacc as bacc
    import concourse.mybir as mybir
    import concourse.tile as tile
    from concourse.masks import make_identity

    f32 = mybir.dt.float32
    i16 = mybir.dt.int16
    TOTC, TOTI, MAXCH = meta["TOTC"], meta["TOTI"], meta["MAXCH"]
    calls = meta["calls"]
    blk_tot_ch = meta["blk_tot_ch"]

    nc = bacc.Bacc("TRN2", num_devices=NCORES)

    xs0 = nc.dram_tensor("xs0", [NPAD, H], f32, kind="ExternalInput")
    x0l = nc.dram_tensor("x0l", [NSP, H], f32, kind="ExternalInput")
    idx_in = nc.dram_tensor("idx", [P, TOTI], i16, kind="ExternalInput")
    dstl_in = nc.dram_tensor("dstl", [P, TOTC], f32, kind="ExternalInput")
    iota_in = nc.dram_tensor("iota", [P, P], f32, kind="ExternalInput")
    dinv_in = nc.dram_tensor("dinv", [P, NBLK], f32, kind="ExternalInput")
    w_in = nc.dram_tensor("wcat", [H, 15 * H], f32, kind="ExternalInput")
    b_in = nc.dram_tensor("bcat", [1, 5 * H], f32, kind="ExternalInput")
    ones_in = nc.dram_tensor("ones", [1, P], f32, kind="ExternalInput")
    xout = nc.dram_tensor("xout", [NLAYERS, NSP, H], f32,
                          kind="ExternalOutput")

    # internal DRAM
    tables = [nc.dram_tensor(f"table{k}", [NPAD, H], f32, kind="Internal",
                             addr_space="Shared") for k in range(NLAYERS - 1)]
    xs_in = [nc.dram_tensor(f"xsin{k}", [NSP, H], f32, kind="Internal")
             for k in range(NLAYERS - 1)]
    aggT_d = [nc.dram_tensor(f"aggT{k}", [H, NSP], f32, kind="Internal")
              for k in range(NLAYERS - 1)]

    def woff(i, j):
        return ((i - 1) * i // 2 + j) * H

    with tile.TileContext(nc) as tc:
        with (
            tc.tile_pool(name="const", bufs=1) as cpool,
            tc.tile_pool(name="xbuf", bufs=1) as xpool,
            tc.tile_pool(name="msgs", bufs=3) as mpool,
            tc.tile_pool(name="ohp", bufs=8) as opool,
            tc.tile_pool(name="work", bufs=4) as wpool,
            tc.tile_pool(name="aggtp", bufs=4) as apool,
            tc.tile_pool(name="aggps", bufs=5, space="PSUM") as agg_ps,
            tc.tile_pool(name="dps", bufs=2, space="PSUM") as dense_ps,
            tc.tile_pool(name="tps", bufs=1, space="PSUM") as tp_ps,
        ):
            # resident constants
            idx_t = cpool.tile([P, TOTI], i16)
            nc.sync.dma_start(out=idx_t[:], in_=idx_in[:])
            dstl_t = cpool.tile([P, TOTC], f32)
            nc.sync.dma_start(out=dstl_t[:], in_=dstl_in[:])
            iota_t = cpool.tile([P, P], f32)
            nc.sync.dma_start(out=iota_t[:], in_=iota_in[:])
            dinv_t = cpool.tile([P, NBLK], f32)
            nc.sync.dma_start(out=dinv_t[:], in_=dinv_in[:])
            w_t = cpool.tile([H, 15 * H], f32)
            nc.sync.dma_start(out=w_t[:], in_=w_in[:])
            b_t = cpool.tile([1, 5 * H], f32)
            nc.sync.dma_start(out=b_t[:], in_=b_in[:])
            ones_t = cpool.tile([1, P], f32)
            nc.sync.dma_start(out=ones_t[:], in_=ones_in[:])
            ident_t = cpool.tile([P, P], f32)
            make_identity(nc, ident_t[:])

            # resident x (ping-pong) [128, NBLK, 64]
            xa = xpool.tile([P, NBLK, H], f32, name="xa")
            xb = xpool.tile([P, NBLK, H], f32, name="xb")
            nc.sync.dma_start(
                out=xa[:], in_=x0l.rearrange("(b p) h -> p b h", p=P))
            xbufs = [xa, xb]

            # aggT tiles of the current layer kept for the dense combine
            for li in range(1, NLAYERS + 1):
                xprev = xbufs[(li - 1) % 2]
                xnext = xbufs[li % 2]
                if li == 1:
                    table = xs0
                else:
                    k = li - 2
                    nc.gpsimd.collective_compute(
                        "AllGather",
                        mybir.AluOpType.bypass,
                        replica_groups=[list(range(NCORES))],
                        ins=[xs_in[k][:].opt()],
                        outs=[tables[k][:].opt()],
                    )
                    table = tables[k]

                blk_done = {}
                psums = {}
                sb_blocks = {}
                for cl in calls:
                    for b, nb in cl["blocks"]:
                        sb_blocks.setdefault(b // GBLK, set()).add(b)

                def close_block(b, li=li, xprev=xprev, xnext=xnext):
                    """dinv scale + transpose + aggT + dense + epilogue."""
                    psum = psums.pop(b)
                    agg_sb = wpool.tile([P, H], f32, tag="aggsb")
                    nc.vector.tensor_scalar(
                        out=agg_sb[:], in0=psum[:],
                        scalar1=dinv_t[:, b:b + 1], scalar2=None,
                        op0=mybir.AluOpType.mult)
                    psT = tp_ps.tile([H, P], f32, space="PSUM", tag="psT")
                    nc.tensor.transpose(
                        out=psT[:], in_=agg_sb[:], identity=ident_t[:])
                    aggT_sb = apool.tile([H, P], f32, tag="aggTsb")
                    nc.scalar.copy(out=aggT_sb[:], in_=psT[:])
                    if li < NLAYERS:
                        nc.sync.dma_start(
                            out=aggT_d[li - 1][:, b * P:(b + 1) * P],
                            in_=aggT_sb[:])
                    # dense combine
                    pd = dense_ps.tile([P, H], f32, space="PSUM", tag="pd")
                    nc.tensor.matmul(
                        pd[:], lhsT=ones_t[:1, :], rhs=b_t[:1, (li - 1) * H:li * H],
                        start=True, stop=False, skip_group_check=True)
                    for j in range(li - 1):
                        lt = apool.tile([H, P], f32, tag="lt")
                        nc.scalar.dma_start(
                            out=lt[:], in_=aggT_d[j][:, b * P:(b + 1) * P])
                        nc.tensor.matmul(
                            pd[:], lhsT=lt[:], rhs=w_t[:, woff(li, j):woff(li, j) + H],
                            start=False, stop=False, skip_group_check=True)
                    nc.tensor.matmul(
                        pd[:], lhsT=aggT_sb[:],
                        rhs=w_t[:, woff(li, li - 1):woff(li, li - 1) + H],
                        start=False, stop=True, skip_group_check=True)
                    # x_new = relu(x_prev + pd)
                    nc.vector.tensor_tensor(
                        out=xnext[:, b, :], in0=pd[:], in1=xprev[:, b, :],
                        op=mybir.AluOpType.add)
                    nc.vector.tensor_scalar(
                        out=xnext[:, b, :], in0=xnext[:, b, :],
                        scalar1=0.0, scalar2=None,
                        op0=mybir.AluOpType.max)
                    nc.sync.dma_start(
                        out=xout[li - 1, b * P:(b + 1) * P, :],
                        in_=xnext[:, b, :])
                    if li < NLAYERS:
                        xs_sb = wpool.tile([P, H], f32, tag="xssb")
                        nc.vector.tensor_scalar(
                            out=xs_sb[:], in0=xnext[:, b, :],
                            scalar1=dinv_t[:, b:b + 1], scalar2=None,
                            op0=mybir.AluOpType.mult)
                        nc.sync.dma_start(
                            out=xs_in[li - 1][b * P:(b + 1) * P, :],
                            in_=xs_sb[:])

                cur_sb = -1
                for cl in calls:
                    qq, nch, io, co = (cl["q"], cl["nch"], cl["idx_off"],
                                       cl["ch_off"])
                    sb = cl["blocks"][0][0] // GBLK
                    if sb != cur_sb:
                        if cur_sb >= 0:
                            for b in sorted(sb_blocks[cur_sb]):
                                close_block(b)
                        cur_sb = sb
                    ncall = nch * P
                    qhi = min((qq + 1) * QROWS, NPAD)
                    msg = mpool.tile([P, MAXCH, H], f32, tag="msg")
                    nc.gpsimd.dma_gather(
                        msg[:, :nch, :], table[qq * QROWS:qhi, :],
                        idx_t[:, io:io + ncall // 16], ncall, ncall, H)
                    ci = 0
                    for b, nb in cl["blocks"]:
                        if b not in psums:
                            psums[b] = agg_ps.tile([P, H], f32, space="PSUM",
                                                   tag="ps", name=f"ps{b}")
                            blk_done[b] = 0
                        psum = psums[b]
                        tot = int(blk_tot_ch[b])
                        for c in range(nb):
                            oh = opool.tile([P, P], f32, tag="oh")
                            nc.vector.tensor_scalar(
                                out=oh[:], in0=iota_t[:],
                                scalar1=dstl_t[:, co + ci:co + ci + 1],
                                scalar2=None,
                                op0=mybir.AluOpType.is_equal)
                            nc.tensor.matmul(
                                psum[:], lhsT=oh[:], rhs=msg[:, ci, :],
                                start=(blk_done[b] == 0),
                                stop=(blk_done[b] == tot - 1),
                                skip_group_check=True)
                            blk_done[b] += 1
                            ci += 1
                for b in sorted(sb_blocks[cur_sb]):
                    close_block(b)

    nc.compile()
    return nc


# ----------------------------------------------------------------------------
# public entry point
# ----------------------------------------------------------------------------

def prepare(inputs):
    """Preprocess graph, compile (cached), and build per-core input maps."""
    global _COMPILED
    x = np.asarray(inputs["x"], dtype=np.float32)
    edge_index = np.asarray(inputs["edge_index"])
    Ws = [np.asarray(inputs[f"W{i}"], dtype=np.float32) for i in range(1, 6)]
    bs = [np.asarray(inputs[f"b{i}"], dtype=np.float32) for i in range(1, 6)]

    meta, idx_all, dstl_all, dinv, _, _ = _preprocess(edge_index)

    if _COMPILED is None:
        _COMPILED = _build_nc(meta)
    nc = _COMPILED

    # host-side input packing
    xs0 = np.zeros((NPAD, H), np.float32)
    xs_full = x * dinv[:, None]
    for c in range(NCORES):
        xs0[c * NSP:c * NSP + NS] = xs_full[c * NS:(c + 1) * NS]
    iota = np.tile(np.arange(P, dtype=np.float32), (P, 1))
    # stored as [H(k-part of each 64-block), 15*H]: layer i block j at woff
    wcat2 = np.zeros((H, 15 * H), np.float32)
    off = 0
    for i in range(1, 6):
        for j in range(i):
            wcat2[:, off:off + H] = Ws[i - 1][j * H:(j + 1) * H, :]
            off += H
    bcat = np.concatenate(bs)[None, :]  # [1, 5*H]
    ones = np.ones((1, P), np.float32)

    in_maps = []
    for c in range(NCORES):
        x0l = np.zeros((NSP, H), np.float32)
        x0l[:NS] = x[c * NS:(c + 1) * NS]
        dinv_l = np.zeros((P, NBLK), np.float32)
        dv = np.zeros(NSP, np.float32)
        dv[:NS] = dinv[c * NS:(c + 1) * NS]
        dinv_l[:, :] = dv.reshape(NBLK, P).T
        in_maps.append({
            "xs0": xs0, "x0l": x0l, "idx": idx_all[c], "dstl": dstl_all[c],
            "iota": iota, "dinv": dinv_l, "wcat": wcat2, "bcat": bcat,
            "ones": ones,
        })
    return nc, in_maps, x


def kernel(**inputs):
    nc, in_maps, x = prepare(inputs)

    from concourse.bass_utils import run_bass_kernel_spmd
    res = run_bass_kernel_spmd(nc, in_maps, core_ids=list(range(NCORES)))

    out = np.empty((N, 6 * H), np.float32)
    out[:, :H] = x
    for c in range(NCORES):
        xo = res.results[c]["xout"]  # [5, NSP, H]
        for li in range(NLAYERS):
            out[c * NS:(c + 1) * NS, (li + 1) * H:(li + 2) * H] = xo[li, :NS]
    return out



# revision 12
# speedup vs baseline: 1.3501x; 1.3501x over previous
"""GCN message-passing kernel for Trainium2 (8 NeuronCores, SPMD) — v2.

Problem: 5-layer GCN with densenet-style concat over a random graph
(N=100000 nodes, E=1600000 edges + self-loops, H=64).

v2 structure (nodes sharded by destination across 8 cores):
  - x resident in SBUF as [128, 98 blocks, 64]; node l lives at
    (partition l%128, block l//128).
  - Gather tables are partition-major: table row c*12544 + (l%128)*98 +
    l//128 holds xs = x*dinv of node (c, l). Layer 1 uses a host-built
    table; later layers AllGather xs shards (one 3.2MB DMA per layer,
    contiguous per partition) into Internal Shared tables.
  - Per 128-edge chunk: one-hot [128e, 128d] built on DVE (8 chunks per
    tensor_tensor via broadcast APs), segment-sum via TensorE matmul into
    per-block PSUM.
  - Per block: dinv scale, PE transpose, aggT cached in SBUF as bf16
    (packed two layers per [128, 98, 128] tile via partition halves;
    no DRAM round-trips), dense combine in bf16 (W resident bf16),
    residual + relu on DVE, xs shard written to a resident tile.
  - xout written once per layer ([128, 6272] f32, 1 DMA); host reassembles.

Host side does graph preprocessing only: self-loops, degree/dinv,
edge sort + padding to 128-edge chunks, int16 index packing.
"""

import numpy as np
import ml_dtypes

BF16 = ml_dtypes.bfloat16

N = 100000
E_RAW = 1600000
H = 64
NCORES = 8
NS = 12500            # real nodes per core
NSP = 12544           # padded (98 * 128)
NPAD = NSP * NCORES   # 100352 padded global rows
P = 128
NBLK = NSP // P       # 98
QROWS = 32768         # rows addressable by int16 idx per gather call
NQ = 4
GBLK = 4              # blocks per superblock (PSUM-bank limited)
NSB = (NBLK + GBLK - 1) // GBLK  # 25
NLAYERS = 5
MAXCALL = 8           # chunks per dma_gather call (1024-desc SWDGE ring)
OHB = 8               # one-hot chunks per DVE tensor_tensor

_COMPILED = None  # (nc, meta) cache


# ----------------------------------------------------------------------------
# host-side graph preprocessing
# ----------------------------------------------------------------------------

def _preprocess(edge_index):
    src = np.asarray(edge_index[0], dtype=np.int64)
    dst = np.asarray(edge_index[1], dtype=np.int64)
    loop = np.arange(N, dtype=np.int64)
    src = np.concatenate([src, loop])
    dst = np.concatenate([dst, loop])

    deg = np.bincount(dst, minlength=N).astype(np.float32)
    dinv = np.where(deg > 0, 1.0 / np.sqrt(np.maximum(deg, 1e-12)), 0.0)
    dinv = dinv.astype(np.float32)

    # partition-major table row for each (padded) source node
    c_src = src // NS
    l_src = src % NS
    rtab = c_src * NSP + (l_src % P) * NBLK + (l_src // P)

    per_core = []
    for c in range(NCORES):
        m = (dst >= c * NS) & (dst < (c + 1) * NS)
        dl = (dst[m] - c * NS).astype(np.int64)
        rt = rtab[m]
        blk = dl // P
        q = rt // QROWS
        sb = blk // GBLK
        order = np.lexsort((blk, q, sb))
        per_core.append((dl[order], rt[order], blk[order], q[order]))

    # per-(b,q) edge counts per core -> uniform capacities (in chunks)
    counts = np.zeros((NCORES, NBLK, NQ), np.int64)
    for c in range(NCORES):
        dl, rt, blk, q = per_core[c]
        np.add.at(counts[c], (blk, q), 1)
    cap = counts.max(axis=0)                      # [NBLK, NQ] edges
    capc = (cap + P - 1) // P                     # chunks

    # static call/chunk schedule (same for every core)
    calls = []
    idx_off = 0
    ch_off = 0
    for s in range(NSB):
        bs = range(s * GBLK, min((s + 1) * GBLK, NBLK))
        for qq in range(NQ):
            pend = [(b, int(capc[b, qq])) for b in bs if capc[b, qq] > 0]
            while pend:
                blocks = []
                room = MAXCALL
                while pend and room > 0:
                    b, nb = pend[0]
                    take = min(nb, room)
                    blocks.append((b, take))
                    room -= take
                    if take == nb:
                        pend.pop(0)
                    else:
                        pend[0] = (b, nb - take)
                nch = sum(nb for _, nb in blocks)
                calls.append(dict(q=qq, nch=nch, idx_off=idx_off,
                                  ch_off=ch_off, blocks=blocks))
                idx_off += nch * P // 16
                ch_off += nch
    TOTC = ch_off
    TOTC8 = ((TOTC + OHB - 1) // OHB) * OHB
    TOTI = idx_off
    MAXCH = max(cl["nch"] for cl in calls)

    blk_tot_ch = capc.sum(axis=1)  # chunks per block

    # pack per-core idx / dstl
    idx_all = np.zeros((NCORES, P, TOTI), np.int16)
    dstl_all = np.full((NCORES, P, TOTC8), 200.0, np.float32)
    for c in range(NCORES):
        dl, rt, blk, q = per_core[c]
        key = blk * NQ + q
        grp_idx = {}
        bounds = np.flatnonzero(np.diff(key)) + 1
        starts = np.concatenate([[0], bounds])
        ends = np.concatenate([bounds, [len(key)]])
        for st, en in zip(starts, ends):
            grp_idx[(int(blk[st]), int(q[st]))] = (st, en)
        padded = {}
        for b in range(NBLK):
            for qq in range(NQ):
                nb = int(capc[b, qq])
                if nb == 0:
                    continue
                st, en = grp_idx.get((b, qq), (0, 0))
                n = en - st
                rel = np.zeros(nb * P, np.int64)
                dd = np.full(nb * P, 200, np.int64)
                if n:
                    rel[:n] = rt[st:en] - qq * QROWS
                    dd[:n] = dl[st:en] - b * P
                padded[(b, qq)] = (rel, dd)
        cursor = {}
        for cl in calls:
            qq = cl["q"]
            io = cl["idx_off"]
            co = cl["ch_off"]
            rel_parts = []
            dst_parts = []
            for b, nb in cl["blocks"]:
                cur = cursor.get((b, qq), 0)
                rel, dd = padded[(b, qq)]
                rel_parts.append(rel[cur * P:(cur + nb) * P])
                dst_parts.append(dd[cur * P:(cur + nb) * P])
                cursor[(b, qq)] = cur + nb
            rel = np.concatenate(rel_parts)
            dd = np.concatenate(dst_parts)
            ncall = cl["nch"] * P
            w = rel.astype(np.int16).reshape(-1, 16).T
            idx_all[c, :, io:io + ncall // 16] = np.tile(w, (8, 1))
            dstl_all[c, :, co:co + cl["nch"]] = (
                dd.astype(np.float32).reshape(-1, P).T)

    meta = dict(calls=calls, TOTC=TOTC, TOTC8=TOTC8, TOTI=TOTI, MAXCH=MAXCH,
                blk_tot_ch=blk_tot_ch)
    return meta, idx_all, dstl_all, dinv


# ----------------------------------------------------------------------------
# device kernel builder
# ----------------------------------------------------------------------------

def _build_nc(meta):
    import concourse.bacc as bacc
    import concourse.mybir as mybir
    import concourse.tile as tile

    f32 = mybir.dt.float32
    bf16 = mybir.dt.bfloat16
    i16 = mybir.dt.int16
    TOTC, TOTC8, TOTI, MAXCH = (meta["TOTC"], meta["TOTC8"], meta["TOTI"],
                                meta["MAXCH"])
    calls = meta["calls"]
    blk_tot_ch = meta["blk_tot_ch"]
    bias_zero = meta["bias_zero"]

    nc = bacc.Bacc("TRN2", num_devices=NCORES)

    xs0 = nc.dram_tensor("xs0", [NPAD, H], f32, kind="ExternalInput")
    x0l = nc.dram_tensor("x0l", [P, NBLK * H], f32, kind="ExternalInput")
    idx_in = nc.dram_tensor("idx", [P, TOTI], i16, kind="ExternalInput")
    dstl_in = nc.dram_tensor("dstl", [P, TOTC8], f32, kind="ExternalInput")
    iota_in = nc.dram_tensor("iota", [P, P], f32, kind="ExternalInput")
    dinv_in = nc.dram_tensor("dinv", [P, NBLK], f32, kind="ExternalInput")
    w_in = nc.dram_tensor("wcat", [P, 15 * H], bf16, kind="ExternalInput")
    b_in = nc.dram_tensor("bcat", [1, 5 * H], bf16, kind="ExternalInput")
    ones_in = nc.dram_tensor("ones", [1, P], bf16, kind="ExternalInput")
    xout = nc.dram_tensor("xout", [NLAYERS, P, NBLK * H], f32,
                          kind="ExternalOutput")

    tables = [nc.dram_tensor(f"table{k}", [NPAD, H], f32, kind="Internal",
                             addr_space="Shared") for k in range(NLAYERS - 1)]
    xs_in = [nc.dram_tensor(f"xsin{k}", [NSP, H], f32, kind="Internal")
             for k in range(NLAYERS - 1)]

    def woff(i, j):
        return ((i - 1) * i // 2 + j) * H

    with tile.TileContext(nc) as tc:
        with (
            tc.tile_pool(name="const", bufs=1) as cpool,
            tc.tile_pool(name="xbuf", bufs=1) as xpool,
            tc.tile_pool(name="msgs", bufs=3) as mpool,
            tc.tile_pool(name="ohp", bufs=3) as opool,
            tc.tile_pool(name="work", bufs=4) as wpool,
            tc.tile_pool(name="aggps", bufs=3, space="PSUM") as agg_ps,
            tc.tile_pool(name="dps", bufs=2, space="PSUM") as dense_ps,
        ):
            # resident constants
            idx_t = cpool.tile([P, TOTI], i16)
            nc.sync.dma_start(out=idx_t[:], in_=idx_in[:])
            dstl_t = cpool.tile([P, TOTC8], f32)
            nc.sync.dma_start(out=dstl_t[:], in_=dstl_in[:])
            iota_t = cpool.tile([P, P], f32)
            nc.sync.dma_start(out=iota_t[:], in_=iota_in[:])
            dinv_t = cpool.tile([P, NBLK], f32)
            nc.sync.dma_start(out=dinv_t[:], in_=dinv_in[:])
            w_t = cpool.tile([P, 15 * H], bf16)
            nc.sync.dma_start(out=w_t[:], in_=w_in[:])
            b_t = cpool.tile([1, 5 * H], bf16)
            nc.sync.dma_start(out=b_t[:], in_=b_in[:])
            ones_t = cpool.tile([1, P], bf16)
            nc.sync.dma_start(out=ones_t[:], in_=ones_in[:])

            # resident x (ping-pong), xs shard, aggT caches.
            # aggT layer k, block b lives at partitions [64*(b%2), +64),
            # free slot b//2 of aggT_L[k]  (UNSCALED segment sums; the dst
            # dinv scale commutes through the dense combine and is applied
            # in the epilogue).
            xa = xpool.tile([P, NBLK, H], f32, name="xa")
            xb = xpool.tile([P, NBLK, H], f32, name="xb")
            xs_sb = xpool.tile([P, NBLK, H], f32, name="xs_sb")
            aggT_L = [xpool.tile([P, NBLK // 2, P], bf16, name=f"aggT{k}")
                      for k in range(NLAYERS - 1)]
            nc.sync.dma_start(
                out=xa[:].rearrange("p b h -> p (b h)"), in_=x0l[:])
            xbufs = [xa, xb]

            def aggT_slice(k, b):
                # layer k (0-based) block b -> [64, 128] bf16 lhsT view
                lo = 64 * (b % 2)
                return aggT_L[k][lo:lo + 64, b // 2, :]

            for li in range(1, NLAYERS + 1):
                xprev = xbufs[(li - 1) % 2]
                xnext = xbufs[li % 2]
                if li == 1:
                    table = xs0
                else:
                    k = li - 2
                    nc.gpsimd.collective_compute(
                        "AllGather",
                        mybir.AluOpType.bypass,
                        replica_groups=[list(range(NCORES))],
                        ins=[xs_in[k][:].opt()],
                        outs=[tables[k][:].opt()],
                    )
                    table = tables[k]

                blk_done = {}
                psums = {}
                sb_blocks = {}
                for cl in calls:
                    for b, nb in cl["blocks"]:
                        sb_blocks.setdefault(b // GBLK, set()).add(b)

                pair_closed = {}

                def close_block(b, li=li, xprev=xprev, xnext=xnext):
                    """aggT cache copy + dense combine + fused epilogue."""
                    pr = b // 2
                    lo = 64 * (b % 2)
                    psum = psums[pr]
                    if pair_closed.get(pr):
                        del psums[pr]
                    pair_closed[pr] = True
                    # cache the (unscaled) aggT block; partitions preserved
                    if li < NLAYERS:
                        aggT_cur = aggT_slice(li - 1, b)
                    else:
                        t5 = wpool.tile([P, P], bf16, tag="aggT5")
                        aggT_cur = t5[lo:lo + 64, :]
                    nc.scalar.copy(out=aggT_cur, in_=psum[lo:lo + 64, :])
                    # dense combine (bf16); W replicated in both partition
                    # halves so rhs base_partition matches the aggT slice
                    pd = dense_ps.tile([P, H], f32, space="PSUM", tag="pd")
                    for j in range(li):
                        lhsT = aggT_cur if j == li - 1 else aggT_slice(j, b)
                        nc.tensor.matmul(
                            pd[:], lhsT=lhsT,
                            rhs=w_t[lo:lo + 64,
                                    woff(li, j):woff(li, j) + H],
                            start=(j == 0), stop=(j == li - 1),
                            skip_group_check=True)
                    # x_new = relu(x_prev + dinv*pd [+ b])
                    nc.vector.scalar_tensor_tensor(
                        out=xnext[:, b, :], in0=pd[:],
                        scalar=dinv_t[:, b:b + 1], in1=xprev[:, b, :],
                        op0=mybir.AluOpType.mult, op1=mybir.AluOpType.add)
                    if not bias_zero:
                        nc.vector.tensor_tensor(
                            out=xnext[:, b, :], in0=xnext[:, b, :],
                            in1=b_t[:1, (li - 1) * H:li * H]
                                .partition_broadcast(P),
                            op=mybir.AluOpType.add)
                    nc.vector.tensor_scalar(
                        out=xnext[:, b, :], in0=xnext[:, b, :],
                        scalar1=0.0, scalar2=None,
                        op0=mybir.AluOpType.max)
                    if li < NLAYERS:
                        nc.vector.tensor_scalar(
                            out=xs_sb[:, b, :], in0=xnext[:, b, :],
                            scalar1=dinv_t[:, b:b + 1], scalar2=None,
                            op0=mybir.AluOpType.mult)

                oh_tile = [None]

                def get_oh(gc):
                    """one-hot slice [128, 128] for global chunk gc; builds
                    a batch of OHB chunks on first use."""
                    g0 = (gc // OHB) * OHB
                    if oh_tile[0] is None or oh_tile[0][0] != g0:
                        oh8 = opool.tile([P, OHB, P], f32, tag="oh8")
                        nc.vector.tensor_tensor(
                            out=oh8[:],
                            in0=dstl_t[:, g0:g0 + OHB]
                                .rearrange("p (k a) -> p k a", a=1)
                                .to_broadcast([P, OHB, P]),
                            in1=iota_t[:]
                                .rearrange("p (a x) -> p a x", a=1)
                                .to_broadcast([P, OHB, P]),
                            op=mybir.AluOpType.is_equal)
                        oh_tile[0] = (g0, oh8)
                    return oh_tile[0][1][:, gc - oh_tile[0][0], :]

                cur_sb = -1
                for cl in calls:
                    qq, nch, io, co = (cl["q"], cl["nch"], cl["idx_off"],
                                       cl["ch_off"])
                    sb = cl["blocks"][0][0] // GBLK
                    if sb != cur_sb:
                        if cur_sb >= 0:
                            for b in sorted(sb_blocks[cur_sb]):
                                close_block(b)
                        cur_sb = sb
                    ncall = nch * P
                    qhi = min((qq + 1) * QROWS, NPAD)
                    msg = mpool.tile([P, MAXCH, H], f32, tag="msg")
                    nc.gpsimd.dma_gather(
                        msg[:, :nch, :], table[qq * QROWS:qhi, :],
                        idx_t[:, io:io + ncall // 16], ncall, ncall, H)
                    ci = 0
                    for b, nb in cl["blocks"]:
                        pr = b // 2
                        lo = 64 * (b % 2)
                        if pr not in psums:
                            psums[pr] = agg_ps.tile([P, P], f32,
                                                    space="PSUM", tag="ps",
                                                    name=f"ps{pr}")
                        if b not in blk_done:
                            blk_done[b] = 0
                        psum = psums[pr]
                        tot = int(blk_tot_ch[b])
                        for c in range(nb):
                            nc.tensor.matmul(
                                psum[lo:lo + 64, :],
                                lhsT=msg[:, ci, :],
                                rhs=get_oh(co + ci),
                                start=(blk_done[b] == 0),
                                stop=(blk_done[b] == tot - 1),
                                skip_group_check=True)
                            blk_done[b] += 1
                            ci += 1
                for b in sorted(sb_blocks[cur_sb]):
                    close_block(b)

                # layer-level stores (one DMA each)
                nc.sync.dma_start(
                    out=xout[li - 1],
                    in_=xnext[:].rearrange("p b h -> p (b h)"))
                if li < NLAYERS:
                    nc.sync.dma_start(
                        out=xs_in[li - 1][:].rearrange("(p b) h -> p b h",
                                                       p=P),
                        in_=xs_sb[:])

    nc.compile()
    return nc


# ----------------------------------------------------------------------------
# public entry point
# ----------------------------------------------------------------------------

def prepare(inputs):
    """Preprocess graph, compile (cached), and build per-core input maps."""
    global _COMPILED
    x = np.asarray(inputs["x"], dtype=np.float32)
    edge_index = np.asarray(inputs["edge_index"])
    Ws = [np.asarray(inputs[f"W{i}"], dtype=np.float32) for i in range(1, 6)]
    bs = [np.asarray(inputs[f"b{i}"], dtype=np.float32) for i in range(1, 6)]

    meta, idx_all, dstl_all, dinv = _preprocess(edge_index)
    meta["bias_zero"] = all(not np.any(b) for b in bs)

    if _COMPILED is None:
        _COMPILED = _build_nc(meta)
    nc = _COMPILED

    # host-side input packing (partition-major node layout)
    xs_full = x * dinv[:, None]
    xs0 = np.zeros((NPAD, H), np.float32)
    x0l_all = np.zeros((NCORES, P, NBLK, H), np.float32)
    dinv_all = np.zeros((NCORES, P, NBLK), np.float32)
    for c in range(NCORES):
        xc = np.zeros((NSP, H), np.float32)
        xc[:NS] = x[c * NS:(c + 1) * NS]
        xsc = np.zeros((NSP, H), np.float32)
        xsc[:NS] = xs_full[c * NS:(c + 1) * NS]
        dvc = np.zeros(NSP, np.float32)
        dvc[:NS] = dinv[c * NS:(c + 1) * NS]
        # node l -> (partition l%128, block l//128)
        x0l_all[c] = xc.reshape(NBLK, P, H).transpose(1, 0, 2)
        dinv_all[c] = dvc.reshape(NBLK, P).T
        # table row = p*NBLK + b
        xs0[c * NSP:(c + 1) * NSP] = (
            xsc.reshape(NBLK, P, H).transpose(1, 0, 2).reshape(NSP, H))
    iota = np.tile(np.arange(P, dtype=np.float32), (P, 1))
    wcat2 = np.zeros((P, 15 * H), np.float32)
    off = 0
    for i in range(1, 6):
        for j in range(i):
            wcat2[:H, off:off + H] = Ws[i - 1][j * H:(j + 1) * H, :]
            off += H
    wcat2[H:] = wcat2[:H]
    wcat2 = wcat2.astype(BF16)
    bcat = np.concatenate(bs)[None, :].astype(BF16)
    ones = np.ones((1, P), BF16)

    in_maps = []
    for c in range(NCORES):
        in_maps.append({
            "xs0": xs0,
            "x0l": x0l_all[c].reshape(P, NBLK * H),
            "idx": idx_all[c], "dstl": dstl_all[c],
            "iota": iota, "dinv": dinv_all[c], "wcat": wcat2, "bcat": bcat,
            "ones": ones,
        })
    return nc, in_maps, x


def kernel(**inputs):
    nc, in_maps, x = prepare(inputs)

    from concourse.bass_utils import run_bass_kernel_spmd
    res = run_bass_kernel_spmd(nc, in_maps, core_ids=list(range(NCORES)))

    out = np.empty((N, 6 * H), np.float32)
    out[:, :H] = x
    for c in range(NCORES):
        xo = res.results[c]["xout"]  # [5, 128, 98*64]
        xo = xo.reshape(NLAYERS, P, NBLK, H).transpose(0, 2, 1, 3)
        xo = xo.reshape(NLAYERS, NSP, H)
        for li in range(NLAYERS):
            out[c * NS:(c + 1) * NS, (li + 1) * H:(li + 2) * H] = xo[li, :NS]
    return out
